# revision 15
# baseline (speedup 1.0000x reference)
"""GCN (2-layer, mean/add/min/max aggregation) Trainium2 Bass kernel, 8 NeuronCores.

v3: table-free edge gather + fused pipeline. Nodes partitioned by destination
across 8 cores (5000/core, one degree-sorted phase of 40 x 128-lane blocks).
Per layer each core computes g = dinv * (h @ W.T) for its shard in both
node-major (bf16 -> gsh -> AllGather -> DRAM gfull, double-buffered per layer)
and feature-major (SBUF gT, used as the self-loop message). Non-self edge
messages are gathered feature-major straight from DRAM (dma_gather
transpose=True, no SBUF staging table). The int16 gather-index limit (<32768)
is handled with two overlapping source windows A=[0,32768) and B=[8192,40960):
each dest's edges split between two message buffers, balanced ~deg/2 per side
inside each 128-lane block to keep slot padding low. Per block both sides are
segment-reduced (add f32 with exact pad correction, min/max bf16). Per 512-lane
group, side combining + self fold (gT) + dinv scaling + the 512->128 combine
matmul (bf16) + bias/ReLU + the next layer's g matmuls (or the final logits
with constant-shift log_softmax) are emitted as soon as that group's chunks
land, so they hide under the Pool-engine descriptor generation that dominates
the kernel. The AllGather is split in two lane-piece collectives that fire
under the previous layer's gather tail (gfull is double-buffered to avoid the
WAR serialization).
"""
import sys

sys.path.insert(0, "/opt/trn_rl_repo")

import numpy as np
from contextlib import ExitStack

import concourse.bacc as bacc
import concourse.tile as tile
import concourse.mybir as mybir
from concourse import bass_utils

N = 40000
E = 640000
D = 128
NCLS = 40
CORES = 8
NPC = N // CORES            # 5000 nodes/core
NPADC = 5120                # padded nodes/core (40 blocks of 128 lanes)
BLK = NPADC // 128          # 40 blocks
NG = CORES * NPADC          # 40960 global g rows
WIN = 32768                 # int16 window size
BOFF = NG - WIN             # 8192: window B covers [8192, 40960)
PIECES = 10
PSZ = NPADC // PIECES      # 512 lanes per AllGather piece
MSG_COLS = 6144
GRP = 512                   # lanes per fused combine/E/A group
NGRP = NPADC // GRP         # 10 groups


def _wrap_idx(idx):
    """int16 -> [128, n/16] wrapped (i -> [i%16, i//16]) and replicated x8."""
    idx = np.asarray(idx, dtype=np.int16)
    n = len(idx)
    assert n % 16 == 0
    cols = n // 16
    base = np.zeros((16, cols), dtype=np.int16)
    base[np.arange(n) % 16, np.arange(n) // 16] = idx
    return np.tile(base, (8, 1))


def _host_prep(x, edge_index):
    # deg/dinv include the appended self-loops (as in the reference)
    row = np.asarray(edge_index[0]).astype(np.int64)   # E original edges only
    col = np.asarray(edge_index[1]).astype(np.int64)
    deg = (np.bincount(col, minlength=N) + 1).astype(np.float64)
    dinv = deg ** -0.5
    ddeg = dinv / deg

    # per-core degree-sorted lane order; gpos = global row in gfull
    # (piece-major layout: (c, lane) -> (lane//PSZ)*8*PSZ + c*PSZ + lane%PSZ)
    lane_of_node = np.zeros(N, dtype=np.int64)
    node_of_lane = np.full((CORES, NPADC), -1, dtype=np.int64)
    for c in range(CORES):
        degs_c = deg[c * NPC:(c + 1) * NPC]
        o = np.argsort(-degs_c, kind="stable")
        lane_of_node[c * NPC + o] = np.arange(NPC)
        node_of_lane[c, :NPC] = c * NPC + o
    lane_all = lane_of_node.copy()
    core_all = np.repeat(np.arange(CORES), NPC)
    gpos = (lane_all // PSZ) * CORES * PSZ + core_all * PSZ + (lane_all % PSZ)

    # per-core non-self edge lists sorted by (lane, side-category)
    per_core_edges = []
    mA_all = np.zeros((CORES, NPADC), dtype=np.int64)
    mB_all = np.zeros((CORES, NPADC), dtype=np.int64)
    cnt_all = np.zeros((CORES, NPADC), dtype=np.int64)
    for c in range(CORES):
        sel = (col >= c * NPC) & (col < (c + 1) * NPC)
        lanes = lane_of_node[col[sel]]
        gp = gpos[row[sel]]
        cat = np.ones(len(gp), dtype=np.int64)          # free
        cat[gp < BOFF] = 0                              # must-A
        cat[gp >= WIN] = 2                              # must-B
        sidx = np.lexsort((cat, lanes))
        lanes, gp, cat = lanes[sidx], gp[sidx], cat[sidx]
        cnt = np.bincount(lanes, minlength=NPADC)
        off = np.zeros(NPADC + 1, dtype=np.int64)
        off[1:] = np.cumsum(cnt)
        mA_all[c] = np.bincount(lanes[cat == 0], minlength=NPADC)
        mB_all[c] = np.bincount(lanes[cat == 2], minlength=NPADC)
        cnt_all[c] = cnt
        per_core_edges.append((lanes, gp, off, cnt))

    # joint per-block side capacities: S_A + S_B ~ max block degree, with the
    # per-lane must counts respected; the window overlap absorbs the rest
    D_b = cnt_all.reshape(CORES, BLK, 128).max(axis=(0, 2))
    MA_b = mA_all.reshape(CORES, BLK, 128).max(axis=(0, 2))
    MB_b = mB_all.reshape(CORES, BLK, 128).max(axis=(0, 2))
    SA = np.maximum(np.maximum((D_b + 1) // 2, MA_b), 1)
    SB = np.maximum(np.maximum(D_b - SA, MB_b), 1)
    blk_of_lane = np.arange(NPADC) // 128
    nA_all = np.zeros((CORES, NPADC), dtype=np.int64)
    nB_all = np.zeros((CORES, NPADC), dtype=np.int64)
    for c in range(CORES):
        cnt, mA, mB = cnt_all[c], mA_all[c], mB_all[c]
        lo = np.maximum(mA, cnt - SB[blk_of_lane])
        hi = np.minimum(SA[blk_of_lane], cnt - mB)
        assert (lo <= hi).all()
        nA = np.clip((cnt + 1) // 2, lo, hi)
        nB = cnt - nA
        real = cnt > 0
        bad = real & ((nA == 0) | (nB == 0))
        assert not bad.any(), "dest with an unpopulatable gather side"
        nA_all[c], nB_all[c] = nA, nB
    PA = np.zeros(BLK + 1, dtype=np.int64)
    PA[1:] = np.cumsum(128 * SA)
    PB = np.zeros(BLK + 1, dtype=np.int64)
    PB[1:] = np.cumsum(128 * SB)
    colsA, colsB = int(PA[-1]), int(PB[-1])

    per_core = []
    for c in range(CORES):
        lanes, gp, off, cnt = per_core_edges[c]
        nA, nB = nA_all[c], nB_all[c]
        blk = np.arange(NPADC) // 128
        lane_in_blk = np.arange(NPADC) % 128
        baseA = PA[blk] + lane_in_blk * SA[blk]
        baseB = PB[blk] + lane_in_blk * SB[blk]

        rank = np.arange(len(lanes)) - off[lanes]
        isA = rank < nA[lanes]
        posA = baseA[lanes] + rank
        posB = baseB[lanes] + (rank - nA[lanes])
        tokA_real = gp[isA]
        tokB_real = gp[~isA] - BOFF
        assert len(tokA_real) == 0 or (0 <= tokA_real.min() and tokA_real.max() < WIN)
        assert len(tokB_real) == 0 or (0 <= tokB_real.min() and tokB_real.max() < WIN)

        # slot-0 token per lane (pads duplicate it); 0 for empty lanes
        tok0A = np.zeros(NPADC, dtype=np.int64)
        tok0A[lanes[isA & (rank == 0)]] = gp[isA & (rank == 0)]
        tok0B = np.zeros(NPADC, dtype=np.int64)
        firstB = (~isA) & (rank == nA[lanes])
        tok0B[lanes[firstB]] = gp[firstB] - BOFF

        edA = np.zeros(colsA, dtype=np.int64)
        edB = np.zeros(colsB, dtype=np.int64)
        for b in range(BLK):
            lv = slice(b * 128, (b + 1) * 128)
            edA[PA[b]:PA[b + 1]] = np.repeat(tok0A[lv], SA[b])
            edB[PB[b]:PB[b + 1]] = np.repeat(tok0B[lv], SB[b])
        edA[posA[isA]] = tokA_real
        edB[posB[~isA]] = tokB_real

        npadA = (SA[blk] - nA).astype(np.float64)
        npadB = (SB[blk] - nB).astype(np.float64)

        nodes = node_of_lane[c]
        real = nodes >= 0
        gl = np.where(real, nodes, 0)
        xp = np.zeros((NPADC, D), dtype=np.float32)
        xp[real] = np.asarray(x)[gl[real]]
        xT = np.ascontiguousarray(xp.T)
        dinv_l = np.where(real, dinv[gl], 0.0)
        ddeg_l = np.where(real, ddeg[gl], 0.0)

        per_core.append(dict(
            xT=xT,
            dinv_scale=np.ascontiguousarray(
                dinv_l.reshape(BLK, 128).T).astype(np.float32),
            dinvb=np.broadcast_to(dinv_l, (128, NPADC)).astype(np.float32).copy(),
            ddegb=np.broadcast_to(ddeg_l, (128, NPADC)).astype(np.float32).copy(),
            npadbA=np.broadcast_to(npadA, (128, NPADC)).astype(np.float32).copy(),
            npadbB=np.broadcast_to(npadB, (128, NPADC)).astype(np.float32).copy(),
            eidxA=_wrap_idx(edA), eidxB=_wrap_idx(edB),
            real=real, gl=gl,
        ))
    meta = dict(SA=SA, SB=SB, PA=PA, PB=PB, colsA=colsA, colsB=colsB)
    return per_core, meta


def _chunks(S, P, max_cols):
    out, cur, cur_cols = [], [], 0
    for b in range(BLK):
        w = 128 * int(S[b])
        if cur and cur_cols + w > max_cols:
            out.append(cur)
            cur, cur_cols = [], 0
        cur.append(b)
        cur_cols += w
    if cur:
        out.append(cur)
    return out


def _build_program(meta):
    SA, SB, PA, PB = meta["SA"], meta["SB"], meta["PA"], meta["PB"]
    colsA, colsB = meta["colsA"], meta["colsB"]
    f32, bf16, i16 = mybir.dt.float32, mybir.dt.bfloat16, mybir.dt.int16
    AX = mybir.AxisListType.X
    OP = mybir.AluOpType
    AF = mybir.ActivationFunctionType

    nc = bacc.Bacc("TRN2", target_bir_lowering=False, debug=False,
                   num_devices=CORES)
    t_xT = nc.dram_tensor("xT", [128, NPADC], f32, kind="ExternalInput")
    t_w = [nc.dram_tensor(f"W{l}T", [128, 128], f32 if l == 0 else bf16,
                         kind="ExternalInput") for l in range(2)]
    t_c = [nc.dram_tensor(f"C{l}T", [4, 128, 128], bf16, kind="ExternalInput") for l in range(2)]
    t_b = [nc.dram_tensor(f"b{l}", [128, 1], f32, kind="ExternalInput") for l in range(2)]
    t_wout = nc.dram_tensor("WoutT", [128, NCLS], bf16, kind="ExternalInput")
    t_bout4 = nc.dram_tensor("bout4", [128, 4, NCLS], f32, kind="ExternalInput")
    t_dsc = nc.dram_tensor("dinv_scale", [128, BLK], f32, kind="ExternalInput")
    t_dinvb = nc.dram_tensor("dinvb", [128, NPADC], bf16, kind="ExternalInput")
    t_ddegb = nc.dram_tensor("ddegb", [128, NPADC], bf16, kind="ExternalInput")
    t_npadA = nc.dram_tensor("npadbA", [128, NPADC], bf16, kind="ExternalInput")
    t_npadB = nc.dram_tensor("npadbB", [128, NPADC], bf16, kind="ExternalInput")
    t_eidxA = nc.dram_tensor("eidxA", [128, colsA // 16], i16, kind="ExternalInput")
    t_eidxB = nc.dram_tensor("eidxB", [128, colsB // 16], i16, kind="ExternalInput")
    t_out = nc.dram_tensor("out", [NPADC, NCLS], f32, kind="ExternalOutput")
    t_gsh = [nc.dram_tensor(f"gsh{p}", [PSZ, D], bf16, kind="Internal")
             for p in range(PIECES)]
    t_gfull = [nc.dram_tensor(f"gfull{l}", [NG, D], bf16, kind="Internal")
               for l in range(2)]

    chA = _chunks(SA, PA, MSG_COLS)
    chB = _chunks(SB, PB, MSG_COLS)
    # side A first, then side B: the next layer's side-A gathers only need
    # gfull pieces 0-7, which land before the last (side-B-only) pieces
    merged = [("A", ch) for ch in chA] + [("B", ch) for ch in chB]

    with tile.TileContext(nc) as tc, ExitStack() as ctx:
        sb = ctx.enter_context(tc.tile_pool(name="sb", bufs=1))
        lhsp = ctx.enter_context(tc.tile_pool(name="lhsp", bufs=3))
        msgp = ctx.enter_context(tc.tile_pool(name="msgp", bufs=3))
        rhp = ctx.enter_context(tc.tile_pool(name="rhp", bufs=2))
        pg = ctx.enter_context(tc.tile_pool(name="pg", bufs=2, space="PSUM"))
        pc = ctx.enter_context(tc.tile_pool(name="pc", bufs=2, space="PSUM"))
        plg = ctx.enter_context(tc.tile_pool(name="plg", bufs=2, space="PSUM"))

        hT = sb.tile([128, NPADC], bf16, tag="hT")
        gT = sb.tile([128, NPADC], bf16, tag="gT")
        dsc = sb.tile([128, BLK], f32, tag="dsc")
        dinvb = sb.tile([128, NPADC], bf16, tag="dinvb")
        ddegb = sb.tile([128, NPADC], bf16, tag="ddegb")
        npadA = sb.tile([128, NPADC], bf16, tag="npadA")
        npadB = sb.tile([128, NPADC], bf16, tag="npadB")
        eixA = sb.tile([128, colsA // 16], i16, tag="eixA")
        eixB = sb.tile([128, colsB // 16], i16, tag="eixB")
        wout = sb.tile([128, NCLS], bf16, tag="wout")
        bout4 = sb.tile([128, 4, NCLS], f32, tag="bout4")
        wts, cts, bts = [], [], []
        for l in range(2):
            wts.append(sb.tile([128, 128], f32 if l == 0 else bf16,
                                tag=f"wt{l}", name=f"wt{l}"))
            cts.append(sb.tile([128, 4, 128], bf16, tag=f"ct{l}", name=f"ct{l}"))
            bts.append(sb.tile([128, 1], f32, tag=f"bt{l}", name=f"bt{l}"))
        nc.sync.dma_start(wts[0][:], t_w[0].ap())
        nc.sync.dma_start(dsc[:], t_dsc.ap())
        nc.sync.dma_start(dinvb[:], t_dinvb.ap())

        stA_add = sb.tile([128, NPADC], f32, tag="stA_add")
        stB_add = sb.tile([128, NPADC], f32, tag="stB_add")
        stA_mn = sb.tile([128, NPADC], bf16, tag="stA_mn")
        stB_mn = sb.tile([128, NPADC], bf16, tag="stB_mn")
        stA_mx = sb.tile([128, NPADC], bf16, tag="stA_mx")
        stB_mx = sb.tile([128, NPADC], bf16, tag="stB_mx")

        def g_wide(l, jw):
            """g for 512 lanes jw*512..: node-major -> gsh piece, plus
            feature-major gT (the self message) via one wide matmul."""
            wsl = slice(jw * 512, (jw + 1) * 512)
            if l == 0:
                lhs = lhsp.tile([128, 512], f32, tag="lhs")
                nc.sync.dma_start(lhs[:], t_xT.ap()[:, wsl])
                lhs_ap = lhs[:]
            else:
                lhs_ap = hT[:, wsl]
            for k in range(4):
                j = jw * 4 + k
                ps = pg.tile([128, 128], f32, tag="ps_g")
                nc.tensor.matmul(ps[:], lhsT=lhs_ap[:, k * 128:(k + 1) * 128],
                                 rhs=wts[l][:], start=True, stop=True)
                gt = lhsp.tile([128, 128], bf16, tag="gt")
                nc.scalar.activation(gt[:], ps[:], AF.Copy, scale=dsc[:, j:j + 1])
                p = j // (PSZ // 128)
                jj = j - p * (PSZ // 128)
                nc.sync.dma_start(
                    t_gsh[p].ap().rearrange("(a p) d -> p a d", p=128)[:, jj, :],
                    gt[:])
            psT = pg.tile([128, 512], f32, tag="ps_gT")
            nc.tensor.matmul(psT[:], lhsT=wts[l][:], rhs=lhs_ap,
                             start=True, stop=True)
            nc.vector.tensor_tensor(out=gT[:, wsl], in0=psT[:],
                                    in1=dinvb[:, wsl], op=OP.mult)

        def fused_group(l, g):
            """combine + scale + E-matmul for lanes [g*GRP,(g+1)*GRP); then
            next-layer g chunks (l==0) or logits (l==1)."""
            gsl = slice(g * GRP, (g + 1) * GRP)
            nc.vector.tensor_tensor(out=stA_add[:, gsl], in0=stA_add[:, gsl],
                                    in1=stB_add[:, gsl], op=OP.add)
            nc.vector.tensor_tensor(out=stA_mn[:, gsl], in0=stA_mn[:, gsl],
                                    in1=stB_mn[:, gsl], op=OP.min)
            nc.vector.tensor_tensor(out=stA_mx[:, gsl], in0=stA_mx[:, gsl],
                                    in1=stB_mx[:, gsl], op=OP.max)
            # fold in the self-loop message (gT)
            nc.vector.tensor_tensor(out=stA_add[:, gsl], in0=stA_add[:, gsl],
                                    in1=gT[:, gsl], op=OP.add)
            nc.vector.tensor_tensor(out=stA_mn[:, gsl], in0=stA_mn[:, gsl],
                                    in1=gT[:, gsl], op=OP.min)
            nc.vector.tensor_tensor(out=stA_mx[:, gsl], in0=stA_mx[:, gsl],
                                    in1=gT[:, gsl], op=OP.max)
            # scale: mean/add from f32 accumulator; mn/mx in place
            mean_g = rhp.tile([128, GRP], bf16, tag="mean_g")
            add_g = rhp.tile([128, GRP], bf16, tag="add_g")
            nc.vector.tensor_tensor(out=mean_g[:], in0=stA_add[:, gsl],
                                    in1=ddegb[:, gsl], op=OP.mult)
            nc.vector.tensor_tensor(out=add_g[:], in0=stA_add[:, gsl],
                                    in1=dinvb[:, gsl], op=OP.mult)
            nc.vector.tensor_tensor(out=stA_mn[:, gsl], in0=stA_mn[:, gsl],
                                    in1=dinvb[:, gsl], op=OP.mult)
            nc.vector.tensor_tensor(out=stA_mx[:, gsl], in0=stA_mx[:, gsl],
                                    in1=dinvb[:, gsl], op=OP.mult)
            psc = pc.tile([128, GRP], f32, tag="ps_cmb")
            for k, st in enumerate((mean_g[:], add_g[:],
                                    stA_mn[:, gsl], stA_mx[:, gsl])):
                nc.tensor.matmul(psc[:], lhsT=cts[l][:, k, :], rhs=st,
                                 start=(k == 0), stop=(k == 3))
            nc.scalar.activation(hT[:, gsl], psc[:], AF.Relu,
                                 bias=bts[l][:], scale=1.0)
            if l == 0:
                g_wide(1, g)
            else:
                logits_group(g)

        def logits_group(q):
            ps4 = plg.tile([128, 4, NCLS], f32, tag="ps_lg")
            for k in range(4):
                j = q * 4 + k
                nc.tensor.matmul(ps4[:, k, :],
                                 lhsT=hT[:, j * 128:(j + 1) * 128],
                                 rhs=wout[:], start=True, stop=True)
            lg4 = lhsp.tile([128, 4, NCLS], f32, tag="lg4")
            nc.vector.tensor_tensor(out=lg4[:], in0=ps4[:], in1=bout4[:],
                                    op=OP.add)
            ex4 = lhsp.tile([128, 4, NCLS], f32, tag="ex4")
            nc.scalar.activation(ex4[:], lg4[:], AF.Exp)
            se4 = lhsp.tile([128, 4], f32, tag="se4")
            nc.vector.tensor_reduce(out=se4[:], in_=ex4[:], axis=AX, op=OP.add)
            ls4 = lhsp.tile([128, 4], f32, tag="ls4")
            nc.scalar.activation(ls4[:], se4[:], AF.Ln)
            for k in range(4):
                nc.vector.tensor_scalar_sub(lg4[:, k, :], lg4[:, k, :],
                                            ls4[:, k:k + 1])
            nc.sync.dma_start(
                t_out.ap().rearrange("(a p) n -> p a n", p=128)[:, 4 * q:4 * q + 4, :],
                lg4[:])

        def ag_piece(p, dst):
            nc.gpsimd.collective_compute(
                "AllGather", OP.bypass, replica_groups=[list(range(CORES))],
                ins=[t_gsh[p].ap()],
                outs=[t_gfull[dst].ap()[p * CORES * PSZ:(p + 1) * CORES * PSZ, :]])

        # ---- layer 0 A-stage from xT, AllGather pieces as they complete
        for p in range(PIECES):
            for jw in range(p * (PSZ // 512), (p + 1) * (PSZ // 512)):
                g_wide(0, jw)
            ag_piece(p, 0)

        # non-critical loads: after the startup A-chain so they don't delay it
        nc.sync.dma_start(eixA[:], t_eidxA.ap())
        nc.sync.dma_start(eixB[:], t_eidxB.ap())
        nc.sync.dma_start(ddegb[:], t_ddegb.ap())
        nc.sync.dma_start(npadA[:], t_npadA.ap())
        nc.sync.dma_start(npadB[:], t_npadB.ap())
        nc.sync.dma_start(wts[1][:], t_w[1].ap())
        for l in range(2):
            nc.sync.dma_start(cts[l][:], t_c[l].ap().rearrange("k p f -> p k f"))
            nc.sync.dma_start(bts[l][:], t_b[l].ap())
        nc.sync.dma_start(wout[:], t_wout.ap())
        nc.sync.dma_start(bout4[:], t_bout4.ap())

        for l in range(2):
            # ---- gathers + reduces, fused groups as blocks complete
            covA = np.zeros(BLK, dtype=bool)
            covB = np.zeros(BLK, dtype=bool)
            next_g = 0
            ag_next = 0
            for side, ch in merged:
                S, P, eix, lo, hi = (
                    (SA, PA, eixA, 0, WIN) if side == "A"
                    else (SB, PB, eixB, BOFF, NG))
                st_add = stA_add if side == "A" else stB_add
                st_mn = stA_mn if side == "A" else stB_mn
                st_mx = stA_mx if side == "A" else stB_mx
                npadS = npadA if side == "A" else npadB
                q0 = int(P[ch[0]])
                qn = int(P[ch[-1] + 1]) - q0
                msg = msgp.tile([128, 1, MSG_COLS], bf16, tag="msg")
                nc.gpsimd.dma_gather(
                    out_ap=msg[:, :, :qn],
                    in_ap=t_gfull[l].ap()[lo:hi, :],
                    idxs_ap=eix[:, q0 // 16:(q0 + qn) // 16],
                    num_idxs=qn, num_idxs_reg=qn, elem_size=D,
                    transpose=True, single_packet=False)
                for b in ch:
                    sbl = int(S[b])
                    cb = int(P[b]) - q0
                    view = msg[:, 0, cb:cb + 128 * sbl].rearrange(
                        "p (l s) -> p l s", s=sbl)
                    dsl = slice(b * 128, (b + 1) * 128)
                    nc.vector.tensor_reduce(
                        out=st_add[:, dsl], in_=view, axis=AX, op=OP.add)
                    nc.vector.tensor_reduce(
                        out=st_mn[:, dsl], in_=view, axis=AX, op=OP.min)
                    nc.vector.tensor_reduce(
                        out=st_mx[:, dsl], in_=view, axis=AX, op=OP.max)
                    tmp = lhsp.tile([128, 128], f32, tag="tmp")
                    nc.vector.tensor_tensor(
                        out=tmp[:], in0=view[:, :, 0], in1=npadS[:, dsl],
                        op=OP.mult)
                    nc.vector.tensor_tensor(
                        out=st_add[:, dsl], in0=st_add[:, dsl],
                        in1=tmp[:], op=OP.subtract)
                    if side == "A":
                        covA[b] = True
                    else:
                        covB[b] = True
                while next_g < NGRP and covA[next_g * 4:(next_g + 1) * 4].all() \
                        and covB[next_g * 4:(next_g + 1) * 4].all():
                    fused_group(l, next_g)
                    next_g += 1
                # fire layer-1 AllGather pieces once their lanes (+1 group
                # of slack so the Pool never stalls on them) are through E/A
                if l == 0:
                    while ag_next < PIECES - 1 and next_g >= (
                            ((ag_next + 1) * PSZ + GRP - 1) // GRP + 1):
                        ag_piece(ag_next, 1)
                        ag_next += 1
            assert next_g == NGRP
            if l == 0:
                for p in range(ag_next, PIECES):
                    ag_piece(p, 1)

    nc.compile()
    return nc


_CACHE = {}


def kernel(x, edge_index, W0, C0, b0, W1, C1, b1, Wout, bout,
           trace=False, _want_results=False):
    x = np.asarray(x, dtype=np.float32)
    per_core, meta = _host_prep(x, edge_index)
    key = (tuple(meta["SA"]), tuple(meta["SB"]))
    if key not in _CACHE:
        _CACHE[key] = _build_program(meta)
    nc = _CACHE[key]

    import ml_dtypes
    shared = dict(
        W0T=np.ascontiguousarray(np.asarray(W0, np.float32).T),
        W1T=np.ascontiguousarray(np.asarray(W1, np.float32).T).astype(ml_dtypes.bfloat16),
        C0T=np.ascontiguousarray(np.asarray(C0, np.float32).T).reshape(4, 128, 128).astype(ml_dtypes.bfloat16),
        C1T=np.ascontiguousarray(np.asarray(C1, np.float32).T).reshape(4, 128, 128).astype(ml_dtypes.bfloat16),
        b0=np.asarray(b0, np.float32).reshape(128, 1),
        b1=np.asarray(b1, np.float32).reshape(128, 1),
        WoutT=np.ascontiguousarray(np.asarray(Wout, np.float32).T).astype(ml_dtypes.bfloat16),
        bout4=np.broadcast_to(np.asarray(bout, np.float32), (128, 4, NCLS)).copy(),
    )
    in_maps = []
    for c in range(CORES):
        d = per_core[c]
        m = dict(shared)
        m.update(xT=d["xT"], dinv_scale=d["dinv_scale"],
                 dinvb=d["dinvb"].astype(ml_dtypes.bfloat16),
                 ddegb=d["ddegb"].astype(ml_dtypes.bfloat16),
                 npadbA=d["npadbA"].astype(ml_dtypes.bfloat16),
                 npadbB=d["npadbB"].astype(ml_dtypes.bfloat16),
                 eidxA=d["eidxA"], eidxB=d["eidxB"])
        in_maps.append(m)

    res = bass_utils.run_bass_kernel_spmd(
        nc, in_maps, core_ids=list(range(CORES)), trace=trace)

    out = np.zeros((N, NCLS), dtype=np.float32)
    for c in range(CORES):
        o = res.results[c]["out"]
        d = per_core[c]
        out[d["gl"][d["real"]]] = o[d["real"]]
    if _want_results:
        return out, res
    return out


# revision 16
# speedup vs baseline: 1.0984x; 1.0984x over previous
"""GCN (2-layer, mean/add/min/max aggregation) Trainium2 Bass kernel, 8 NeuronCores.

v3: table-free edge gather + fused pipeline. Nodes partitioned by destination
across 8 cores (5000/core, one degree-sorted phase of 40 x 128-lane blocks).
Per layer each core computes g = dinv * (h @ W.T) for its shard in both
node-major (bf16 -> gsh -> AllGather -> DRAM gfull, double-buffered per layer)
and feature-major (SBUF gT, used as the self-loop message). Non-self edge
messages are gathered feature-major straight from DRAM (dma_gather
transpose=True, no SBUF staging table). The int16 gather-index limit (<32768)
is handled with two overlapping source windows A=[0,32768) and B=[8192,40960):
each dest's edges split between two message buffers, balanced ~deg/2 per side
inside each 128-lane block to keep slot padding low. Per block both sides are
segment-reduced (add f32 with exact pad correction, min/max bf16). Per 512-lane
group, side combining + self fold (gT) + dinv scaling + the 512->128 combine
matmul (bf16) + bias/ReLU + the next layer's g matmuls (or the final logits
with constant-shift log_softmax) are emitted as soon as that group's chunks
land, so they hide under the Pool-engine descriptor generation that dominates
the kernel. The AllGather is split in two lane-piece collectives that fire
under the previous layer's gather tail (gfull is double-buffered to avoid the
WAR serialization).
"""
import sys

sys.path.insert(0, "/opt/trn_rl_repo")

import numpy as np
from contextlib import ExitStack

import concourse.bacc as bacc
import concourse.tile as tile
import concourse.mybir as mybir
from concourse import bass_utils

N = 40000
E = 640000
D = 128
NCLS = 40
CORES = 8
NPC = N // CORES            # 5000 nodes/core
NPADC = 5120                # padded nodes/core (40 blocks of 128 lanes)
BLK = NPADC // 128          # 40 blocks
NG = CORES * NPADC          # 40960 global g rows
WIN = 32768                 # int16 window size
BOFF = NG - WIN             # 8192: window B covers [8192, 40960)
PIECES = 5
PSZ = NPADC // PIECES       # 1024 lanes per AllGather piece
MSG_COLS = 6144
GRP = 512                   # lanes per fused combine/E/A group
NGRP = NPADC // GRP         # 10 groups


def _wrap_idx(idx):
    """int16 -> [128, n/16] wrapped (i -> [i%16, i//16]) and replicated x8."""
    idx = np.asarray(idx, dtype=np.int16)
    n = len(idx)
    assert n % 16 == 0
    cols = n // 16
    base = np.zeros((16, cols), dtype=np.int16)
    base[np.arange(n) % 16, np.arange(n) // 16] = idx
    return np.tile(base, (8, 1))


def _host_prep(x, edge_index):
    # deg/dinv include the appended self-loops (as in the reference)
    row = np.asarray(edge_index[0]).astype(np.int64)   # E original edges only
    col = np.asarray(edge_index[1]).astype(np.int64)
    deg = (np.bincount(col, minlength=N) + 1).astype(np.float64)
    dinv = deg ** -0.5
    ddeg = dinv / deg

    # per-core degree-sorted lane order; gpos = global row in gfull
    # (piece-major layout: (c, lane) -> (lane//PSZ)*8*PSZ + c*PSZ + lane%PSZ)
    lane_of_node = np.zeros(N, dtype=np.int64)
    node_of_lane = np.full((CORES, NPADC), -1, dtype=np.int64)
    for c in range(CORES):
        degs_c = deg[c * NPC:(c + 1) * NPC]
        o = np.argsort(-degs_c, kind="stable")
        lane_of_node[c * NPC + o] = np.arange(NPC)
        node_of_lane[c, :NPC] = c * NPC + o
    lane_all = lane_of_node.copy()
    core_all = np.repeat(np.arange(CORES), NPC)
    gpos = (lane_all // PSZ) * CORES * PSZ + core_all * PSZ + (lane_all % PSZ)

    # per-core non-self edge lists sorted by (lane, side-category)
    per_core_edges = []
    mA_all = np.zeros((CORES, NPADC), dtype=np.int64)
    mB_all = np.zeros((CORES, NPADC), dtype=np.int64)
    cnt_all = np.zeros((CORES, NPADC), dtype=np.int64)
    for c in range(CORES):
        sel = (col >= c * NPC) & (col < (c + 1) * NPC)
        lanes = lane_of_node[col[sel]]
        gp = gpos[row[sel]]
        cat = np.ones(len(gp), dtype=np.int64)          # free
        cat[gp < BOFF] = 0                              # must-A
        cat[gp >= WIN] = 2                              # must-B
        sidx = np.lexsort((cat, lanes))
        lanes, gp, cat = lanes[sidx], gp[sidx], cat[sidx]
        cnt = np.bincount(lanes, minlength=NPADC)
        off = np.zeros(NPADC + 1, dtype=np.int64)
        off[1:] = np.cumsum(cnt)
        mA_all[c] = np.bincount(lanes[cat == 0], minlength=NPADC)
        mB_all[c] = np.bincount(lanes[cat == 2], minlength=NPADC)
        cnt_all[c] = cnt
        per_core_edges.append((lanes, gp, off, cnt))

    # joint per-block side capacities: S_A + S_B ~ max block degree, with the
    # per-lane must counts respected; the window overlap absorbs the rest
    D_b = cnt_all.reshape(CORES, BLK, 128).max(axis=(0, 2))
    MA_b = mA_all.reshape(CORES, BLK, 128).max(axis=(0, 2))
    MB_b = mB_all.reshape(CORES, BLK, 128).max(axis=(0, 2))
    SA = np.maximum(np.maximum((D_b + 1) // 2, MA_b), 1)
    SB = np.maximum(np.maximum(D_b - SA, MB_b), 1)
    blk_of_lane = np.arange(NPADC) // 128
    nA_all = np.zeros((CORES, NPADC), dtype=np.int64)
    nB_all = np.zeros((CORES, NPADC), dtype=np.int64)
    for c in range(CORES):
        cnt, mA, mB = cnt_all[c], mA_all[c], mB_all[c]
        lo = np.maximum(mA, cnt - SB[blk_of_lane])
        hi = np.minimum(SA[blk_of_lane], cnt - mB)
        assert (lo <= hi).all()
        nA = np.clip((cnt + 1) // 2, lo, hi)
        nB = cnt - nA
        real = cnt > 0
        bad = real & ((nA == 0) | (nB == 0))
        assert not bad.any(), "dest with an unpopulatable gather side"
        nA_all[c], nB_all[c] = nA, nB
    PA = np.zeros(BLK + 1, dtype=np.int64)
    PA[1:] = np.cumsum(128 * SA)
    PB = np.zeros(BLK + 1, dtype=np.int64)
    PB[1:] = np.cumsum(128 * SB)
    colsA, colsB = int(PA[-1]), int(PB[-1])

    per_core = []
    for c in range(CORES):
        lanes, gp, off, cnt = per_core_edges[c]
        nA, nB = nA_all[c], nB_all[c]
        blk = np.arange(NPADC) // 128
        lane_in_blk = np.arange(NPADC) % 128
        baseA = PA[blk] + lane_in_blk * SA[blk]
        baseB = PB[blk] + lane_in_blk * SB[blk]

        rank = np.arange(len(lanes)) - off[lanes]
        isA = rank < nA[lanes]
        posA = baseA[lanes] + rank
        posB = baseB[lanes] + (rank - nA[lanes])
        tokA_real = gp[isA]
        tokB_real = gp[~isA] - BOFF
        assert len(tokA_real) == 0 or (0 <= tokA_real.min() and tokA_real.max() < WIN)
        assert len(tokB_real) == 0 or (0 <= tokB_real.min() and tokB_real.max() < WIN)

        # slot-0 token per lane (pads duplicate it); 0 for empty lanes
        tok0A = np.zeros(NPADC, dtype=np.int64)
        tok0A[lanes[isA & (rank == 0)]] = gp[isA & (rank == 0)]
        tok0B = np.zeros(NPADC, dtype=np.int64)
        firstB = (~isA) & (rank == nA[lanes])
        tok0B[lanes[firstB]] = gp[firstB] - BOFF

        edA = np.zeros(colsA, dtype=np.int64)
        edB = np.zeros(colsB, dtype=np.int64)
        for b in range(BLK):
            lv = slice(b * 128, (b + 1) * 128)
            edA[PA[b]:PA[b + 1]] = np.repeat(tok0A[lv], SA[b])
            edB[PB[b]:PB[b + 1]] = np.repeat(tok0B[lv], SB[b])
        edA[posA[isA]] = tokA_real
        edB[posB[~isA]] = tokB_real

        npadA = (SA[blk] - nA).astype(np.float64)
        npadB = (SB[blk] - nB).astype(np.float64)

        nodes = node_of_lane[c]
        real = nodes >= 0
        gl = np.where(real, nodes, 0)
        xp = np.zeros((NPADC, D), dtype=np.float32)
        xp[real] = np.asarray(x)[gl[real]]
        xT = np.ascontiguousarray(xp.T)
        dinv_l = np.where(real, dinv[gl], 0.0)
        ddeg_l = np.where(real, ddeg[gl], 0.0)

        per_core.append(dict(
            xT=xT,
            dinv_scale=np.ascontiguousarray(
                dinv_l.reshape(BLK, 128).T).astype(np.float32),
            dinvb=np.broadcast_to(dinv_l, (128, NPADC)).astype(np.float32).copy(),
            ddegb=np.broadcast_to(ddeg_l, (128, NPADC)).astype(np.float32).copy(),
            npadbA=np.broadcast_to(npadA, (128, NPADC)).astype(np.float32).copy(),
            npadbB=np.broadcast_to(npadB, (128, NPADC)).astype(np.float32).copy(),
            eidxA=_wrap_idx(edA), eidxB=_wrap_idx(edB),
            real=real, gl=gl,
        ))
    meta = dict(SA=SA, SB=SB, PA=PA, PB=PB, colsA=colsA, colsB=colsB)
    return per_core, meta


def _chunks(S, P, max_cols):
    out, cur, cur_cols = [], [], 0
    for b in range(BLK):
        w = 128 * int(S[b])
        if cur and cur_cols + w > max_cols:
            out.append(cur)
            cur, cur_cols = [], 0
        cur.append(b)
        cur_cols += w
    if cur:
        out.append(cur)
    return out


def _build_program(meta):
    SA, SB, PA, PB = meta["SA"], meta["SB"], meta["PA"], meta["PB"]
    colsA, colsB = meta["colsA"], meta["colsB"]
    f32, bf16, i16 = mybir.dt.float32, mybir.dt.bfloat16, mybir.dt.int16
    AX = mybir.AxisListType.X
    OP = mybir.AluOpType
    AF = mybir.ActivationFunctionType

    nc = bacc.Bacc("TRN2", target_bir_lowering=False, debug=False,
                   num_devices=CORES)
    t_xT = nc.dram_tensor("xT", [128, NPADC], f32, kind="ExternalInput")
    t_w = [nc.dram_tensor(f"W{l}T", [128, 128], f32 if l == 0 else bf16,
                         kind="ExternalInput") for l in range(2)]
    t_c = [nc.dram_tensor(f"C{l}T", [4, 128, 128], bf16, kind="ExternalInput") for l in range(2)]
    t_b = [nc.dram_tensor(f"b{l}", [128, 1], f32, kind="ExternalInput") for l in range(2)]
    t_wout = nc.dram_tensor("WoutT", [128, NCLS], bf16, kind="ExternalInput")
    t_bout4 = nc.dram_tensor("bout4", [128, 4, NCLS], f32, kind="ExternalInput")
    t_dsc = nc.dram_tensor("dinv_scale", [128, BLK], f32, kind="ExternalInput")
    t_dinvb = nc.dram_tensor("dinvb", [128, NPADC], bf16, kind="ExternalInput")
    t_ddegb = nc.dram_tensor("ddegb", [128, NPADC], bf16, kind="ExternalInput")
    t_npadA = nc.dram_tensor("npadbA", [128, NPADC], bf16, kind="ExternalInput")
    t_npadB = nc.dram_tensor("npadbB", [128, NPADC], bf16, kind="ExternalInput")
    t_eidxA = nc.dram_tensor("eidxA", [128, colsA // 16], i16, kind="ExternalInput")
    t_eidxB = nc.dram_tensor("eidxB", [128, colsB // 16], i16, kind="ExternalInput")
    t_out = nc.dram_tensor("out", [NPADC, NCLS], f32, kind="ExternalOutput")
    t_gsh = [nc.dram_tensor(f"gsh{p}", [PSZ, D], bf16, kind="Internal")
             for p in range(PIECES)]
    t_gfull = [nc.dram_tensor(f"gfull{l}", [NG, D], bf16, kind="Internal")
               for l in range(2)]

    chA = _chunks(SA, PA, MSG_COLS)
    chB = _chunks(SB, PB, MSG_COLS)
    # interleave sides by covered block, but give side A a 3-chunk head
    # start: side-A gathers only need gfull pieces 0-3, so the next layer can
    # begin while the last AllGather piece is still landing
    inter = sorted(
        [("A", ch) for ch in chA] + [("B", ch) for ch in chB],
        key=lambda sc: (sc[1][-1], sc[0]))
    LEAD = 3
    a_head = [sc for sc in inter if sc[0] == "A"][:LEAD]
    rest = [sc for sc in inter if sc not in a_head]
    merged = a_head + rest

    with tile.TileContext(nc) as tc, ExitStack() as ctx:
        sb = ctx.enter_context(tc.tile_pool(name="sb", bufs=1))
        lhsp = ctx.enter_context(tc.tile_pool(name="lhsp", bufs=3))
        msgp = ctx.enter_context(tc.tile_pool(name="msgp", bufs=3))
        rhp = ctx.enter_context(tc.tile_pool(name="rhp", bufs=2))
        pg = ctx.enter_context(tc.tile_pool(name="pg", bufs=2, space="PSUM"))
        pc = ctx.enter_context(tc.tile_pool(name="pc", bufs=2, space="PSUM"))
        plg = ctx.enter_context(tc.tile_pool(name="plg", bufs=2, space="PSUM"))

        hT = sb.tile([128, NPADC], bf16, tag="hT")
        gT = sb.tile([128, NPADC], bf16, tag="gT")
        dsc = sb.tile([128, BLK], f32, tag="dsc")
        dinvb = sb.tile([128, NPADC], bf16, tag="dinvb")
        ddegb = sb.tile([128, NPADC], bf16, tag="ddegb")
        npadA = sb.tile([128, NPADC], bf16, tag="npadA")
        npadB = sb.tile([128, NPADC], bf16, tag="npadB")
        eixA = sb.tile([128, colsA // 16], i16, tag="eixA")
        eixB = sb.tile([128, colsB // 16], i16, tag="eixB")
        wout = sb.tile([128, NCLS], bf16, tag="wout")
        bout4 = sb.tile([128, 4, NCLS], f32, tag="bout4")
        wts, cts, bts = [], [], []
        for l in range(2):
            wts.append(sb.tile([128, 128], f32 if l == 0 else bf16,
                                tag=f"wt{l}", name=f"wt{l}"))
            cts.append(sb.tile([128, 4, 128], bf16, tag=f"ct{l}", name=f"ct{l}"))
            bts.append(sb.tile([128, 1], f32, tag=f"bt{l}", name=f"bt{l}"))
        nc.sync.dma_start(wts[0][:], t_w[0].ap())
        nc.sync.dma_start(dsc[:], t_dsc.ap())
        nc.sync.dma_start(dinvb[:], t_dinvb.ap())

        stA_add = sb.tile([128, NPADC], f32, tag="stA_add")
        stB_add = sb.tile([128, NPADC], f32, tag="stB_add")
        stA_mn = sb.tile([128, NPADC], bf16, tag="stA_mn")
        stB_mn = sb.tile([128, NPADC], bf16, tag="stB_mn")
        stA_mx = sb.tile([128, NPADC], bf16, tag="stA_mx")
        stB_mx = sb.tile([128, NPADC], bf16, tag="stB_mx")

        def g_wide(l, jw):
            """g for 512 lanes jw*512..: node-major -> gsh piece, plus
            feature-major gT (the self message) via one wide matmul."""
            wsl = slice(jw * 512, (jw + 1) * 512)
            if l == 0:
                lhs = lhsp.tile([128, 512], f32, tag="lhs")
                nc.sync.dma_start(lhs[:], t_xT.ap()[:, wsl])
                lhs_ap = lhs[:]
            else:
                lhs_ap = hT[:, wsl]
            for k in range(4):
                j = jw * 4 + k
                ps = pg.tile([128, 128], f32, tag="ps_g")
                nc.tensor.matmul(ps[:], lhsT=lhs_ap[:, k * 128:(k + 1) * 128],
                                 rhs=wts[l][:], start=True, stop=True)
                gt = lhsp.tile([128, 128], bf16, tag="gt")
                nc.scalar.activation(gt[:], ps[:], AF.Copy, scale=dsc[:, j:j + 1])
                p = j // (PSZ // 128)
                jj = j - p * (PSZ // 128)
                nc.sync.dma_start(
                    t_gsh[p].ap().rearrange("(a p) d -> p a d", p=128)[:, jj, :],
                    gt[:])
            psT = pg.tile([128, 512], f32, tag="ps_gT")
            nc.tensor.matmul(psT[:], lhsT=wts[l][:], rhs=lhs_ap,
                             start=True, stop=True)
            nc.vector.tensor_tensor(out=gT[:, wsl], in0=psT[:],
                                    in1=dinvb[:, wsl], op=OP.mult)

        def fused_group(l, g):
            """combine + scale + E-matmul for lanes [g*GRP,(g+1)*GRP); then
            next-layer g chunks (l==0) or logits (l==1)."""
            gsl = slice(g * GRP, (g + 1) * GRP)
            nc.vector.tensor_tensor(out=stA_add[:, gsl], in0=stA_add[:, gsl],
                                    in1=stB_add[:, gsl], op=OP.add)
            nc.vector.tensor_tensor(out=stA_mn[:, gsl], in0=stA_mn[:, gsl],
                                    in1=stB_mn[:, gsl], op=OP.min)
            nc.vector.tensor_tensor(out=stA_mx[:, gsl], in0=stA_mx[:, gsl],
                                    in1=stB_mx[:, gsl], op=OP.max)
            # fold in the self-loop message (gT)
            nc.vector.tensor_tensor(out=stA_add[:, gsl], in0=stA_add[:, gsl],
                                    in1=gT[:, gsl], op=OP.add)
            nc.vector.tensor_tensor(out=stA_mn[:, gsl], in0=stA_mn[:, gsl],
                                    in1=gT[:, gsl], op=OP.min)
            nc.vector.tensor_tensor(out=stA_mx[:, gsl], in0=stA_mx[:, gsl],
                                    in1=gT[:, gsl], op=OP.max)
            # scale: mean/add from f32 accumulator; mn/mx in place
            mean_g = rhp.tile([128, GRP], bf16, tag="mean_g")
            add_g = rhp.tile([128, GRP], bf16, tag="add_g")
            nc.vector.tensor_tensor(out=mean_g[:], in0=stA_add[:, gsl],
                                    in1=ddegb[:, gsl], op=OP.mult)
            nc.vector.tensor_tensor(out=add_g[:], in0=stA_add[:, gsl],
                                    in1=dinvb[:, gsl], op=OP.mult)
            nc.vector.tensor_tensor(out=stA_mn[:, gsl], in0=stA_mn[:, gsl],
                                    in1=dinvb[:, gsl], op=OP.mult)
            nc.vector.tensor_tensor(out=stA_mx[:, gsl], in0=stA_mx[:, gsl],
                                    in1=dinvb[:, gsl], op=OP.mult)
            psc = pc.tile([128, GRP], f32, tag="ps_cmb")
            for k, st in enumerate((mean_g[:], add_g[:],
                                    stA_mn[:, gsl], stA_mx[:, gsl])):
                nc.tensor.matmul(psc[:], lhsT=cts[l][:, k, :], rhs=st,
                                 start=(k == 0), stop=(k == 3))
            nc.scalar.activation(hT[:, gsl], psc[:], AF.Relu,
                                 bias=bts[l][:], scale=1.0)
            if l == 0:
                g_wide(1, g)
            else:
                logits_group(g)

        def logits_group(q):
            ps4 = plg.tile([128, 4, NCLS], f32, tag="ps_lg")
            for k in range(4):
                j = q * 4 + k
                nc.tensor.matmul(ps4[:, k, :],
                                 lhsT=hT[:, j * 128:(j + 1) * 128],
                                 rhs=wout[:], start=True, stop=True)
            lg4 = lhsp.tile([128, 4, NCLS], f32, tag="lg4")
            nc.vector.tensor_tensor(out=lg4[:], in0=ps4[:], in1=bout4[:],
                                    op=OP.add)
            ex4 = lhsp.tile([128, 4, NCLS], f32, tag="ex4")
            nc.scalar.activation(ex4[:], lg4[:], AF.Exp)
            se4 = lhsp.tile([128, 4], f32, tag="se4")
            nc.vector.tensor_reduce(out=se4[:], in_=ex4[:], axis=AX, op=OP.add)
            ls4 = lhsp.tile([128, 4], f32, tag="ls4")
            nc.scalar.activation(ls4[:], se4[:], AF.Ln)
            for k in range(4):
                nc.vector.tensor_scalar_sub(lg4[:, k, :], lg4[:, k, :],
                                            ls4[:, k:k + 1])
            nc.sync.dma_start(
                t_out.ap().rearrange("(a p) n -> p a n", p=128)[:, 4 * q:4 * q + 4, :],
                lg4[:])

        def ag_piece(p, dst):
            nc.gpsimd.collective_compute(
                "AllGather", OP.bypass, replica_groups=[list(range(CORES))],
                ins=[t_gsh[p].ap()],
                outs=[t_gfull[dst].ap()[p * CORES * PSZ:(p + 1) * CORES * PSZ, :]])

        # ---- layer 0 A-stage from xT, AllGather pieces as they complete
        for p in range(PIECES):
            for jw in range(p * (PSZ // 512), (p + 1) * (PSZ // 512)):
                g_wide(0, jw)
            ag_piece(p, 0)

        # non-critical loads: after the startup A-chain so they don't delay it
        nc.sync.dma_start(eixA[:], t_eidxA.ap())
        nc.sync.dma_start(eixB[:], t_eidxB.ap())
        nc.sync.dma_start(ddegb[:], t_ddegb.ap())
        nc.sync.dma_start(npadA[:], t_npadA.ap())
        nc.sync.dma_start(npadB[:], t_npadB.ap())
        nc.sync.dma_start(wts[1][:], t_w[1].ap())
        for l in range(2):
            nc.sync.dma_start(cts[l][:], t_c[l].ap().rearrange("k p f -> p k f"))
            nc.sync.dma_start(bts[l][:], t_b[l].ap())
        nc.sync.dma_start(wout[:], t_wout.ap())
        nc.sync.dma_start(bout4[:], t_bout4.ap())

        for l in range(2):
            # ---- gathers + reduces, fused groups as blocks complete
            covA = np.zeros(BLK, dtype=bool)
            covB = np.zeros(BLK, dtype=bool)
            next_g = 0
            ag_next = 0
            for side, ch in merged:
                S, P, eix, lo, hi = (
                    (SA, PA, eixA, 0, WIN) if side == "A"
                    else (SB, PB, eixB, BOFF, NG))
                st_add = stA_add if side == "A" else stB_add
                st_mn = stA_mn if side == "A" else stB_mn
                st_mx = stA_mx if side == "A" else stB_mx
                npadS = npadA if side == "A" else npadB
                q0 = int(P[ch[0]])
                qn = int(P[ch[-1] + 1]) - q0
                msg = msgp.tile([128, 1, MSG_COLS], bf16, tag="msg")
                nc.gpsimd.dma_gather(
                    out_ap=msg[:, :, :qn],
                    in_ap=t_gfull[l].ap()[lo:hi, :],
                    idxs_ap=eix[:, q0 // 16:(q0 + qn) // 16],
                    num_idxs=qn, num_idxs_reg=qn, elem_size=D,
                    transpose=True, single_packet=False)
                for b in ch:
                    sbl = int(S[b])
                    cb = int(P[b]) - q0
                    view = msg[:, 0, cb:cb + 128 * sbl].rearrange(
                        "p (l s) -> p l s", s=sbl)
                    dsl = slice(b * 128, (b + 1) * 128)
                    nc.vector.tensor_reduce(
                        out=st_add[:, dsl], in_=view, axis=AX, op=OP.add)
                    nc.vector.tensor_reduce(
                        out=st_mn[:, dsl], in_=view, axis=AX, op=OP.min)
                    nc.vector.tensor_reduce(
                        out=st_mx[:, dsl], in_=view, axis=AX, op=OP.max)
                    tmp = lhsp.tile([128, 128], f32, tag="tmp")
                    nc.vector.tensor_tensor(
                        out=tmp[:], in0=view[:, :, 0], in1=npadS[:, dsl],
                        op=OP.mult)
                    nc.vector.tensor_tensor(
                        out=st_add[:, dsl], in0=st_add[:, dsl],
                        in1=tmp[:], op=OP.subtract)
                    if side == "A":
                        covA[b] = True
                    else:
                        covB[b] = True
                while next_g < NGRP and covA[next_g * 4:(next_g + 1) * 4].all() \
                        and covB[next_g * 4:(next_g + 1) * 4].all():
                    fused_group(l, next_g)
                    next_g += 1
                # fire layer-1 AllGather pieces once their lanes (+1 group
                # of slack so the Pool never stalls on them) are through E/A
                if l == 0:
                    while ag_next < PIECES - 1 and next_g >= (
                            ((ag_next + 1) * PSZ + GRP - 1) // GRP + 1):
                        ag_piece(ag_next, 1)
                        ag_next += 1
            assert next_g == NGRP
            if l == 0:
                for p in range(ag_next, PIECES):
                    ag_piece(p, 1)

    nc.compile()
    return nc


_CACHE = {}


def kernel(x, edge_index, W0, C0, b0, W1, C1, b1, Wout, bout,
           trace=False, _want_results=False):
    x = np.asarray(x, dtype=np.float32)
    per_core, meta = _host_prep(x, edge_index)
    key = (tuple(meta["SA"]), tuple(meta["SB"]))
    if key not in _CACHE:
        _CACHE[key] = _build_program(meta)
    nc = _CACHE[key]

    import ml_dtypes
    shared = dict(
        W0T=np.ascontiguousarray(np.asarray(W0, np.float32).T),
        W1T=np.ascontiguousarray(np.asarray(W1, np.float32).T).astype(ml_dtypes.bfloat16),
        C0T=np.ascontiguousarray(np.asarray(C0, np.float32).T).reshape(4, 128, 128).astype(ml_dtypes.bfloat16),
        C1T=np.ascontiguousarray(np.asarray(C1, np.float32).T).reshape(4, 128, 128).astype(ml_dtypes.bfloat16),
        b0=np.asarray(b0, np.float32).reshape(128, 1),
        b1=np.asarray(b1, np.float32).reshape(128, 1),
        WoutT=np.ascontiguousarray(np.asarray(Wout, np.float32).T).astype(ml_dtypes.bfloat16),
        bout4=np.broadcast_to(np.asarray(bout, np.float32), (128, 4, NCLS)).copy(),
    )
    in_maps = []
    for c in range(CORES):
        d = per_core[c]
        m = dict(shared)
        m.update(xT=d["xT"], dinv_scale=d["dinv_scale"],
                 dinvb=d["dinvb"].astype(ml_dtypes.bfloat16),
                 ddegb=d["ddegb"].astype(ml_dtypes.bfloat16),
                 npadbA=d["npadbA"].astype(ml_dtypes.bfloat16),
                 npadbB=d["npadbB"].astype(ml_dtypes.bfloat16),
                 eidxA=d["eidxA"], eidxB=d["eidxB"])
        in_maps.append(m)

    res = bass_utils.run_bass_kernel_spmd(
        nc, in_maps, core_ids=list(range(CORES)), trace=trace)

    out = np.zeros((N, NCLS), dtype=np.float32)
    for c in range(CORES):
        o = res.results[c]["out"]
        d = per_core[c]
        out[d["gl"][d["real"]]] = o[d["real"]]
    if _want_results:
        return out, res
    return out


# revision 17
# speedup vs baseline: 1.1051x; 1.0061x over previous
"""GCN (2-layer, mean/add/min/max aggregation) Trainium2 Bass kernel, 8 NeuronCores.

v3: table-free edge gather + fused pipeline. Nodes partitioned by destination
across 8 cores (5000/core, one degree-sorted phase of 40 x 128-lane blocks).
Per layer each core computes g = dinv * (h @ W.T) for its shard in both
node-major (bf16 -> gsh -> AllGather -> DRAM gfull, double-buffered per layer)
and feature-major (SBUF gT, used as the self-loop message). Non-self edge
messages are gathered feature-major straight from DRAM (dma_gather
transpose=True, no SBUF staging table). The int16 gather-index limit (<32768)
is handled with two overlapping source windows A=[0,32768) and B=[8192,40960):
each dest's edges split between two message buffers, balanced ~deg/2 per side
inside each 128-lane block to keep slot padding low. Per block both sides are
segment-reduced (add f32 with exact pad correction, min/max bf16). Per 512-lane
group, side combining + self fold (gT) + dinv scaling + the 512->128 combine
matmul (bf16) + bias/ReLU + the next layer's g matmuls (or the final logits
with constant-shift log_softmax) are emitted as soon as that group's chunks
land, so they hide under the Pool-engine descriptor generation that dominates
the kernel. The AllGather is split in two lane-piece collectives that fire
under the previous layer's gather tail (gfull is double-buffered to avoid the
WAR serialization).
"""
import sys

sys.path.insert(0, "/opt/trn_rl_repo")

import numpy as np
from contextlib import ExitStack

import concourse.bacc as bacc
import concourse.tile as tile
import concourse.mybir as mybir
from concourse import bass_utils

N = 40000
E = 640000
D = 128
NCLS = 40
CORES = 8
NPC = N // CORES            # 5000 nodes/core
NPADC = 5120                # padded nodes/core (40 blocks of 128 lanes)
BLK = NPADC // 128          # 40 blocks
NG = CORES * NPADC          # 40960 global g rows
WIN = 32768                 # int16 window size
BOFF = NG - WIN             # 8192: window B covers [8192, 40960)
PIECES = 5
PSZ = NPADC // PIECES       # 1024 lanes per AllGather piece
MSG_COLS = 6144
GRP = 512                   # lanes per fused combine/E/A group
NGRP = NPADC // GRP         # 10 groups


def _wrap_idx(idx):
    """int16 -> [128, n/16] wrapped (i -> [i%16, i//16]) and replicated x8."""
    idx = np.asarray(idx, dtype=np.int16)
    n = len(idx)
    assert n % 16 == 0
    cols = n // 16
    base = np.zeros((16, cols), dtype=np.int16)
    base[np.arange(n) % 16, np.arange(n) // 16] = idx
    return np.tile(base, (8, 1))


def _host_prep(x, edge_index):
    # deg/dinv include the appended self-loops (as in the reference)
    row = np.asarray(edge_index[0]).astype(np.int64)   # E original edges only
    col = np.asarray(edge_index[1]).astype(np.int64)
    deg = (np.bincount(col, minlength=N) + 1).astype(np.float64)
    dinv = deg ** -0.5
    ddeg = dinv / deg

    # per-core degree-sorted lane order; gpos = global row in gfull
    # (piece-major layout: (c, lane) -> (lane//PSZ)*8*PSZ + c*PSZ + lane%PSZ)
    lane_of_node = np.zeros(N, dtype=np.int64)
    node_of_lane = np.full((CORES, NPADC), -1, dtype=np.int64)
    for c in range(CORES):
        degs_c = deg[c * NPC:(c + 1) * NPC]
        o = np.argsort(-degs_c, kind="stable")
        lane_of_node[c * NPC + o] = np.arange(NPC)
        node_of_lane[c, :NPC] = c * NPC + o
    lane_all = lane_of_node.copy()
    core_all = np.repeat(np.arange(CORES), NPC)
    gpos = (lane_all // PSZ) * CORES * PSZ + core_all * PSZ + (lane_all % PSZ)

    # per-core non-self edge lists sorted by (lane, side-category)
    per_core_edges = []
    mA_all = np.zeros((CORES, NPADC), dtype=np.int64)
    mB_all = np.zeros((CORES, NPADC), dtype=np.int64)
    cnt_all = np.zeros((CORES, NPADC), dtype=np.int64)
    for c in range(CORES):
        sel = (col >= c * NPC) & (col < (c + 1) * NPC)
        lanes = lane_of_node[col[sel]]
        gp = gpos[row[sel]]
        cat = np.ones(len(gp), dtype=np.int64)          # free
        cat[gp < BOFF] = 0                              # must-A
        cat[gp >= WIN] = 2                              # must-B
        sidx = np.lexsort((cat, lanes))
        lanes, gp, cat = lanes[sidx], gp[sidx], cat[sidx]
        cnt = np.bincount(lanes, minlength=NPADC)
        off = np.zeros(NPADC + 1, dtype=np.int64)
        off[1:] = np.cumsum(cnt)
        mA_all[c] = np.bincount(lanes[cat == 0], minlength=NPADC)
        mB_all[c] = np.bincount(lanes[cat == 2], minlength=NPADC)
        cnt_all[c] = cnt
        per_core_edges.append((lanes, gp, off, cnt))

    # joint per-block side capacities: S_A + S_B ~ max block degree, with the
    # per-lane must counts respected; the window overlap absorbs the rest
    D_b = cnt_all.reshape(CORES, BLK, 128).max(axis=(0, 2))
    MA_b = mA_all.reshape(CORES, BLK, 128).max(axis=(0, 2))
    MB_b = mB_all.reshape(CORES, BLK, 128).max(axis=(0, 2))
    SA = np.maximum(np.maximum((D_b + 1) // 2, MA_b), 1)
    SB = np.maximum(np.maximum(D_b - SA, MB_b), 1)
    blk_of_lane = np.arange(NPADC) // 128
    nA_all = np.zeros((CORES, NPADC), dtype=np.int64)
    nB_all = np.zeros((CORES, NPADC), dtype=np.int64)
    for c in range(CORES):
        cnt, mA, mB = cnt_all[c], mA_all[c], mB_all[c]
        lo = np.maximum(mA, cnt - SB[blk_of_lane])
        hi = np.minimum(SA[blk_of_lane], cnt - mB)
        assert (lo <= hi).all()
        nA = np.clip((cnt + 1) // 2, lo, hi)
        nB = cnt - nA
        real = cnt > 0
        bad = real & ((nA == 0) | (nB == 0))
        assert not bad.any(), "dest with an unpopulatable gather side"
        nA_all[c], nB_all[c] = nA, nB
    PA = np.zeros(BLK + 1, dtype=np.int64)
    PA[1:] = np.cumsum(128 * SA)
    PB = np.zeros(BLK + 1, dtype=np.int64)
    PB[1:] = np.cumsum(128 * SB)
    colsA, colsB = int(PA[-1]), int(PB[-1])

    per_core = []
    for c in range(CORES):
        lanes, gp, off, cnt = per_core_edges[c]
        nA, nB = nA_all[c], nB_all[c]
        blk = np.arange(NPADC) // 128
        lane_in_blk = np.arange(NPADC) % 128
        baseA = PA[blk] + lane_in_blk * SA[blk]
        baseB = PB[blk] + lane_in_blk * SB[blk]

        rank = np.arange(len(lanes)) - off[lanes]
        isA = rank < nA[lanes]
        posA = baseA[lanes] + rank
        posB = baseB[lanes] + (rank - nA[lanes])
        tokA_real = gp[isA]
        tokB_real = gp[~isA] - BOFF
        assert len(tokA_real) == 0 or (0 <= tokA_real.min() and tokA_real.max() < WIN)
        assert len(tokB_real) == 0 or (0 <= tokB_real.min() and tokB_real.max() < WIN)

        # slot-0 token per lane (pads duplicate it); 0 for empty lanes
        tok0A = np.zeros(NPADC, dtype=np.int64)
        tok0A[lanes[isA & (rank == 0)]] = gp[isA & (rank == 0)]
        tok0B = np.zeros(NPADC, dtype=np.int64)
        firstB = (~isA) & (rank == nA[lanes])
        tok0B[lanes[firstB]] = gp[firstB] - BOFF

        edA = np.zeros(colsA, dtype=np.int64)
        edB = np.zeros(colsB, dtype=np.int64)
        for b in range(BLK):
            lv = slice(b * 128, (b + 1) * 128)
            edA[PA[b]:PA[b + 1]] = np.repeat(tok0A[lv], SA[b])
            edB[PB[b]:PB[b + 1]] = np.repeat(tok0B[lv], SB[b])
        edA[posA[isA]] = tokA_real
        edB[posB[~isA]] = tokB_real

        npadA = (SA[blk] - nA).astype(np.float64)
        npadB = (SB[blk] - nB).astype(np.float64)

        nodes = node_of_lane[c]
        real = nodes >= 0
        gl = np.where(real, nodes, 0)
        xp = np.zeros((NPADC, D), dtype=np.float32)
        xp[real] = np.asarray(x)[gl[real]]
        xT = np.ascontiguousarray(xp.T)
        dinv_l = np.where(real, dinv[gl], 0.0)
        ddeg_l = np.where(real, ddeg[gl], 0.0)

        per_core.append(dict(
            xT=xT,
            dinv_scale=np.ascontiguousarray(
                dinv_l.reshape(BLK, 128).T).astype(np.float32),
            dinvb=np.broadcast_to(dinv_l, (128, NPADC)).astype(np.float32).copy(),
            ddegb=np.broadcast_to(ddeg_l, (128, NPADC)).astype(np.float32).copy(),
            npadbA=np.broadcast_to(npadA, (128, NPADC)).astype(np.float32).copy(),
            npadbB=np.broadcast_to(npadB, (128, NPADC)).astype(np.float32).copy(),
            eidxA=_wrap_idx(edA), eidxB=_wrap_idx(edB),
            real=real, gl=gl,
        ))
    meta = dict(SA=SA, SB=SB, PA=PA, PB=PB, colsA=colsA, colsB=colsB)
    return per_core, meta


def _chunks(S, P, max_cols):
    out, cur, cur_cols = [], [], 0
    for b in range(BLK):
        w = 128 * int(S[b])
        if cur and cur_cols + w > max_cols:
            out.append(cur)
            cur, cur_cols = [], 0
        cur.append(b)
        cur_cols += w
    if cur:
        out.append(cur)
    return out


def _build_program(meta):
    SA, SB, PA, PB = meta["SA"], meta["SB"], meta["PA"], meta["PB"]
    colsA, colsB = meta["colsA"], meta["colsB"]
    f32, bf16, i16 = mybir.dt.float32, mybir.dt.bfloat16, mybir.dt.int16
    AX = mybir.AxisListType.X
    OP = mybir.AluOpType
    AF = mybir.ActivationFunctionType

    nc = bacc.Bacc("TRN2", target_bir_lowering=False, debug=False,
                   num_devices=CORES)
    t_xT = nc.dram_tensor("xT", [128, NPADC], bf16, kind="ExternalInput")
    t_w = [nc.dram_tensor(f"W{l}T", [128, 128], bf16,
                         kind="ExternalInput") for l in range(2)]
    t_c = [nc.dram_tensor(f"C{l}T", [4, 128, 128], bf16, kind="ExternalInput") for l in range(2)]
    t_b = [nc.dram_tensor(f"b{l}", [128, 1], f32, kind="ExternalInput") for l in range(2)]
    t_wout = nc.dram_tensor("WoutT", [128, NCLS], bf16, kind="ExternalInput")
    t_bout4 = nc.dram_tensor("bout4", [128, 4, NCLS], f32, kind="ExternalInput")
    t_dsc = nc.dram_tensor("dinv_scale", [128, BLK], f32, kind="ExternalInput")
    t_dinvb = nc.dram_tensor("dinvb", [128, NPADC], bf16, kind="ExternalInput")
    t_ddegb = nc.dram_tensor("ddegb", [128, NPADC], bf16, kind="ExternalInput")
    t_npadA = nc.dram_tensor("npadbA", [128, NPADC], bf16, kind="ExternalInput")
    t_npadB = nc.dram_tensor("npadbB", [128, NPADC], bf16, kind="ExternalInput")
    t_eidxA = nc.dram_tensor("eidxA", [128, colsA // 16], i16, kind="ExternalInput")
    t_eidxB = nc.dram_tensor("eidxB", [128, colsB // 16], i16, kind="ExternalInput")
    t_out = nc.dram_tensor("out", [NPADC, NCLS], f32, kind="ExternalOutput")
    t_gsh = [nc.dram_tensor(f"gsh{p}", [PSZ, D], bf16, kind="Internal")
             for p in range(PIECES)]
    t_gfull = [nc.dram_tensor(f"gfull{l}", [NG, D], bf16, kind="Internal")
               for l in range(2)]

    chA = _chunks(SA, PA, MSG_COLS)
    chB = _chunks(SB, PB, MSG_COLS)
    # interleave sides by covered block, but give side A a 3-chunk head
    # start: side-A gathers only need gfull pieces 0-3, so the next layer can
    # begin while the last AllGather piece is still landing
    inter = sorted(
        [("A", ch) for ch in chA] + [("B", ch) for ch in chB],
        key=lambda sc: (sc[1][-1], sc[0]))
    LEAD = 3
    a_head = [sc for sc in inter if sc[0] == "A"][:LEAD]
    rest = [sc for sc in inter if sc not in a_head]
    merged = a_head + rest

    with tile.TileContext(nc) as tc, ExitStack() as ctx:
        sb = ctx.enter_context(tc.tile_pool(name="sb", bufs=1))
        lhsp = ctx.enter_context(tc.tile_pool(name="lhsp", bufs=3))
        msgp = ctx.enter_context(tc.tile_pool(name="msgp", bufs=3))
        rhp = ctx.enter_context(tc.tile_pool(name="rhp", bufs=2))
        pg = ctx.enter_context(tc.tile_pool(name="pg", bufs=2, space="PSUM"))
        pc = ctx.enter_context(tc.tile_pool(name="pc", bufs=2, space="PSUM"))
        plg = ctx.enter_context(tc.tile_pool(name="plg", bufs=2, space="PSUM"))

        hT = sb.tile([128, NPADC], bf16, tag="hT")
        gT = sb.tile([128, NPADC], bf16, tag="gT")
        dsc = sb.tile([128, BLK], f32, tag="dsc")
        dinvb = sb.tile([128, NPADC], bf16, tag="dinvb")
        ddegb = sb.tile([128, NPADC], bf16, tag="ddegb")
        npadA = sb.tile([128, NPADC], bf16, tag="npadA")
        npadB = sb.tile([128, NPADC], bf16, tag="npadB")
        eixA = sb.tile([128, colsA // 16], i16, tag="eixA")
        eixB = sb.tile([128, colsB // 16], i16, tag="eixB")
        wout = sb.tile([128, NCLS], bf16, tag="wout")
        bout4 = sb.tile([128, 4, NCLS], f32, tag="bout4")
        wts, cts, bts = [], [], []
        for l in range(2):
            wts.append(sb.tile([128, 128], bf16, tag=f"wt{l}", name=f"wt{l}"))
            cts.append(sb.tile([128, 4, 128], bf16, tag=f"ct{l}", name=f"ct{l}"))
            bts.append(sb.tile([128, 1], f32, tag=f"bt{l}", name=f"bt{l}"))
        nc.sync.dma_start(wts[0][:], t_w[0].ap())
        nc.sync.dma_start(dsc[:], t_dsc.ap())
        nc.sync.dma_start(dinvb[:], t_dinvb.ap())

        stA_add = sb.tile([128, NPADC], f32, tag="stA_add")
        stB_add = sb.tile([128, NPADC], f32, tag="stB_add")
        stA_mn = sb.tile([128, NPADC], bf16, tag="stA_mn")
        stB_mn = sb.tile([128, NPADC], bf16, tag="stB_mn")
        stA_mx = sb.tile([128, NPADC], bf16, tag="stA_mx")
        stB_mx = sb.tile([128, NPADC], bf16, tag="stB_mx")

        def g_wide(l, jw):
            """g for 512 lanes jw*512..: node-major -> gsh piece, plus
            feature-major gT (the self message) via one wide matmul."""
            wsl = slice(jw * 512, (jw + 1) * 512)
            if l == 0:
                lhs = lhsp.tile([128, 512], bf16, tag="lhs")
                nc.sync.dma_start(lhs[:], t_xT.ap()[:, wsl])
                lhs_ap = lhs[:]
            else:
                lhs_ap = hT[:, wsl]
            for k in range(4):
                j = jw * 4 + k
                ps = pg.tile([128, 128], f32, tag="ps_g")
                nc.tensor.matmul(ps[:], lhsT=lhs_ap[:, k * 128:(k + 1) * 128],
                                 rhs=wts[l][:], start=True, stop=True)
                gt = lhsp.tile([128, 128], bf16, tag="gt")
                nc.scalar.activation(gt[:], ps[:], AF.Copy, scale=dsc[:, j:j + 1])
                p = j // (PSZ // 128)
                jj = j - p * (PSZ // 128)
                nc.sync.dma_start(
                    t_gsh[p].ap().rearrange("(a p) d -> p a d", p=128)[:, jj, :],
                    gt[:])
            psT = pg.tile([128, 512], f32, tag="ps_gT")
            nc.tensor.matmul(psT[:], lhsT=wts[l][:], rhs=lhs_ap,
                             start=True, stop=True)
            nc.vector.tensor_tensor(out=gT[:, wsl], in0=psT[:],
                                    in1=dinvb[:, wsl], op=OP.mult)

        def fused_group(l, g):
            """combine + scale + E-matmul for lanes [g*GRP,(g+1)*GRP); then
            next-layer g chunks (l==0) or logits (l==1)."""
            gsl = slice(g * GRP, (g + 1) * GRP)
            nc.vector.tensor_tensor(out=stA_add[:, gsl], in0=stA_add[:, gsl],
                                    in1=stB_add[:, gsl], op=OP.add)
            nc.vector.tensor_tensor(out=stA_mn[:, gsl], in0=stA_mn[:, gsl],
                                    in1=stB_mn[:, gsl], op=OP.min)
            nc.vector.tensor_tensor(out=stA_mx[:, gsl], in0=stA_mx[:, gsl],
                                    in1=stB_mx[:, gsl], op=OP.max)
            # fold in the self-loop message (gT)
            nc.vector.tensor_tensor(out=stA_add[:, gsl], in0=stA_add[:, gsl],
                                    in1=gT[:, gsl], op=OP.add)
            nc.vector.tensor_tensor(out=stA_mn[:, gsl], in0=stA_mn[:, gsl],
                                    in1=gT[:, gsl], op=OP.min)
            nc.vector.tensor_tensor(out=stA_mx[:, gsl], in0=stA_mx[:, gsl],
                                    in1=gT[:, gsl], op=OP.max)
            # scale: mean/add from f32 accumulator; mn/mx in place
            mean_g = rhp.tile([128, GRP], bf16, tag="mean_g")
            add_g = rhp.tile([128, GRP], bf16, tag="add_g")
            nc.vector.tensor_tensor(out=mean_g[:], in0=stA_add[:, gsl],
                                    in1=ddegb[:, gsl], op=OP.mult)
            nc.vector.tensor_tensor(out=add_g[:], in0=stA_add[:, gsl],
                                    in1=dinvb[:, gsl], op=OP.mult)
            nc.vector.tensor_tensor(out=stA_mn[:, gsl], in0=stA_mn[:, gsl],
                                    in1=dinvb[:, gsl], op=OP.mult)
            nc.vector.tensor_tensor(out=stA_mx[:, gsl], in0=stA_mx[:, gsl],
                                    in1=dinvb[:, gsl], op=OP.mult)
            psc = pc.tile([128, GRP], f32, tag="ps_cmb")
            for k, st in enumerate((mean_g[:], add_g[:],
                                    stA_mn[:, gsl], stA_mx[:, gsl])):
                nc.tensor.matmul(psc[:], lhsT=cts[l][:, k, :], rhs=st,
                                 start=(k == 0), stop=(k == 3))
            nc.scalar.activation(hT[:, gsl], psc[:], AF.Relu,
                                 bias=bts[l][:], scale=1.0)
            if l == 0:
                g_wide(1, g)
            else:
                logits_group(g)

        def logits_group(q):
            ps4 = plg.tile([128, 4, NCLS], f32, tag="ps_lg")
            for k in range(4):
                j = q * 4 + k
                nc.tensor.matmul(ps4[:, k, :],
                                 lhsT=hT[:, j * 128:(j + 1) * 128],
                                 rhs=wout[:], start=True, stop=True)
            lg4 = lhsp.tile([128, 4, NCLS], f32, tag="lg4")
            nc.vector.tensor_tensor(out=lg4[:], in0=ps4[:], in1=bout4[:],
                                    op=OP.add)
            ex4 = lhsp.tile([128, 4, NCLS], f32, tag="ex4")
            nc.scalar.activation(ex4[:], lg4[:], AF.Exp)
            se4 = lhsp.tile([128, 4], f32, tag="se4")
            nc.vector.tensor_reduce(out=se4[:], in_=ex4[:], axis=AX, op=OP.add)
            ls4 = lhsp.tile([128, 4], f32, tag="ls4")
            nc.scalar.activation(ls4[:], se4[:], AF.Ln)
            for k in range(4):
                nc.vector.tensor_scalar_sub(lg4[:, k, :], lg4[:, k, :],
                                            ls4[:, k:k + 1])
            nc.sync.dma_start(
                t_out.ap().rearrange("(a p) n -> p a n", p=128)[:, 4 * q:4 * q + 4, :],
                lg4[:])

        def ag_piece(p, dst):
            nc.gpsimd.collective_compute(
                "AllGather", OP.bypass, replica_groups=[list(range(CORES))],
                ins=[t_gsh[p].ap()],
                outs=[t_gfull[dst].ap()[p * CORES * PSZ:(p + 1) * CORES * PSZ, :]])

        # ---- layer 0 A-stage from xT, AllGather pieces as they complete
        for p in range(PIECES):
            for jw in range(p * (PSZ // 512), (p + 1) * (PSZ // 512)):
                g_wide(0, jw)
            ag_piece(p, 0)

        # non-critical loads: after the startup A-chain so they don't delay it
        nc.sync.dma_start(eixA[:], t_eidxA.ap())
        nc.sync.dma_start(eixB[:], t_eidxB.ap())
        nc.sync.dma_start(ddegb[:], t_ddegb.ap())
        nc.sync.dma_start(npadA[:], t_npadA.ap())
        nc.sync.dma_start(npadB[:], t_npadB.ap())
        nc.sync.dma_start(wts[1][:], t_w[1].ap())
        for l in range(2):
            nc.sync.dma_start(cts[l][:], t_c[l].ap().rearrange("k p f -> p k f"))
            nc.sync.dma_start(bts[l][:], t_b[l].ap())
        nc.sync.dma_start(wout[:], t_wout.ap())
        nc.sync.dma_start(bout4[:], t_bout4.ap())

        for l in range(2):
            # ---- gathers + reduces, fused groups as blocks complete
            covA = np.zeros(BLK, dtype=bool)
            covB = np.zeros(BLK, dtype=bool)
            next_g = 0
            ag_next = 0
            for side, ch in merged:
                S, P, eix, lo, hi = (
                    (SA, PA, eixA, 0, WIN) if side == "A"
                    else (SB, PB, eixB, BOFF, NG))
                st_add = stA_add if side == "A" else stB_add
                st_mn = stA_mn if side == "A" else stB_mn
                st_mx = stA_mx if side == "A" else stB_mx
                npadS = npadA if side == "A" else npadB
                q0 = int(P[ch[0]])
                qn = int(P[ch[-1] + 1]) - q0
                msg = msgp.tile([128, 1, MSG_COLS], bf16, tag="msg")
                nc.gpsimd.dma_gather(
                    out_ap=msg[:, :, :qn],
                    in_ap=t_gfull[l].ap()[lo:hi, :],
                    idxs_ap=eix[:, q0 // 16:(q0 + qn) // 16],
                    num_idxs=qn, num_idxs_reg=qn, elem_size=D,
                    transpose=True, single_packet=False)
                for b in ch:
                    sbl = int(S[b])
                    cb = int(P[b]) - q0
                    view = msg[:, 0, cb:cb + 128 * sbl].rearrange(
                        "p (l s) -> p l s", s=sbl)
                    dsl = slice(b * 128, (b + 1) * 128)
                    nc.vector.tensor_reduce(
                        out=st_add[:, dsl], in_=view, axis=AX, op=OP.add)
                    nc.vector.tensor_reduce(
                        out=st_mn[:, dsl], in_=view, axis=AX, op=OP.min)
                    nc.vector.tensor_reduce(
                        out=st_mx[:, dsl], in_=view, axis=AX, op=OP.max)
                    tmp = lhsp.tile([128, 128], f32, tag="tmp")
                    nc.vector.tensor_tensor(
                        out=tmp[:], in0=view[:, :, 0], in1=npadS[:, dsl],
                        op=OP.mult)
                    nc.vector.tensor_tensor(
                        out=st_add[:, dsl], in0=st_add[:, dsl],
                        in1=tmp[:], op=OP.subtract)
                    if side == "A":
                        covA[b] = True
                    else:
                        covB[b] = True
                while next_g < NGRP and covA[next_g * 4:(next_g + 1) * 4].all() \
                        and covB[next_g * 4:(next_g + 1) * 4].all():
                    fused_group(l, next_g)
                    next_g += 1
                # fire layer-1 AllGather pieces once their lanes (+1 group
                # of slack so the Pool never stalls on them) are through E/A
                if l == 0:
                    while ag_next < PIECES - 1 and next_g >= (
                            ((ag_next + 1) * PSZ + GRP - 1) // GRP):
                        ag_piece(ag_next, 1)
                        ag_next += 1
            assert next_g == NGRP
            if l == 0:
                for p in range(ag_next, PIECES):
                    ag_piece(p, 1)

    nc.compile()
    return nc


_CACHE = {}


def kernel(x, edge_index, W0, C0, b0, W1, C1, b1, Wout, bout,
           trace=False, _want_results=False):
    x = np.asarray(x, dtype=np.float32)
    per_core, meta = _host_prep(x, edge_index)
    key = (tuple(meta["SA"]), tuple(meta["SB"]))
    if key not in _CACHE:
        _CACHE[key] = _build_program(meta)
    nc = _CACHE[key]

    import ml_dtypes
    shared = dict(
        W0T=np.ascontiguousarray(np.asarray(W0, np.float32).T).astype(ml_dtypes.bfloat16),
        W1T=np.ascontiguousarray(np.asarray(W1, np.float32).T).astype(ml_dtypes.bfloat16),
        C0T=np.ascontiguousarray(np.asarray(C0, np.float32).T).reshape(4, 128, 128).astype(ml_dtypes.bfloat16),
        C1T=np.ascontiguousarray(np.asarray(C1, np.float32).T).reshape(4, 128, 128).astype(ml_dtypes.bfloat16),
        b0=np.asarray(b0, np.float32).reshape(128, 1),
        b1=np.asarray(b1, np.float32).reshape(128, 1),
        WoutT=np.ascontiguousarray(np.asarray(Wout, np.float32).T).astype(ml_dtypes.bfloat16),
        bout4=np.broadcast_to(np.asarray(bout, np.float32), (128, 4, NCLS)).copy(),
    )
    in_maps = []
    for c in range(CORES):
        d = per_core[c]
        m = dict(shared)
        m.update(xT=d["xT"].astype(ml_dtypes.bfloat16),
                 dinv_scale=d["dinv_scale"],
                 dinvb=d["dinvb"].astype(ml_dtypes.bfloat16),
                 ddegb=d["ddegb"].astype(ml_dtypes.bfloat16),
                 npadbA=d["npadbA"].astype(ml_dtypes.bfloat16),
                 npadbB=d["npadbB"].astype(ml_dtypes.bfloat16),
                 eidxA=d["eidxA"], eidxB=d["eidxB"])
        in_maps.append(m)

    res = bass_utils.run_bass_kernel_spmd(
        nc, in_maps, core_ids=list(range(CORES)), trace=trace)

    out = np.zeros((N, NCLS), dtype=np.float32)
    for c in range(CORES):
        o = res.results[c]["out"]
        d = per_core[c]
        out[d["gl"][d["real"]]] = o[d["real"]]
    if _want_results:
        return out, res
    return out
